# revision 14
# baseline (speedup 1.0000x reference)
"""Ragged grouped GEMM (MoE routing) on 8 Trainium2 NeuronCores.

Problem: out[start_g:end_g] = x[start_g:end_g] @ weight[g] for g in 0..7,
with x [16384, 2048] f32, weight [8, 2048, 8192] f32, ragged token counts.

Sharding: 8-way tensor-parallel along DOUT. Each core sees ALL tokens
(pre-transposed x, padded per-expert to 128-token tiles) and a
DOUT/8 = 1024-wide column shard of every expert's weight. The ragged group
structure is identical on every core, so a single SPMD program with
host-hardcoded group boundaries runs on all 8 cores; the host concatenates
the per-core column shards into the full output.

Per-core kernel (Tile framework): x-stationary matmul tiling.
  out_tile[128 tok, 1024] = sum_k xT_tile[k][128 din, 128 tok].T
                                  @ w[g][k][128 din, 1024]
accumulated over 16 k-tiles in PSUM (2 banks), j in {0,1} 512-wide halves.
Weights for the current expert stay SBUF-resident; the next expert's
weights prefetch via tile-pool double buffering.
"""

import os
import sys

import numpy as np

_TRN_REPO = "/opt/trn_rl_repo"

P = 128            # SBUF/PE partition count; token tile and k tile size
CHUNK = 256        # tokens per x DMA chunk (multiple of P)
NSPLIT = 512       # moving free dim per matmul (fp32 max, = 1 PSUM bank)
N_CORES = 8

# "float32":  exact (rel err ~4e-7), 4 cycles/row on PE -> ~4.0 ms.
# "float32r": TF32-like, rel err ~1.4e-4, measured SLOWER than fp32 on HW.
# "bf16x3":   hi/lo bf16 split, 3 cross-products (xh@wh + xh@wl + xl@wh),
#             rel err ~4e-6 (abs ~5e-7, inside the fp32 envelope for
#             K=2048), 6 bf16 matmuls vs fp32's equivalent 8 half-rate
#             passes -> ~25% fewer PE cycles.
MM_DTYPE = os.environ.get("KERNEL_MM_DTYPE", "bf16x3")

_PROG_CACHE = {}
last_run_info = {}


def _concourse():
    if _TRN_REPO not in sys.path:
        sys.path.insert(0, _TRN_REPO)
    import concourse.bass as bass  # noqa: F401
    import concourse.mybir as mybir
    import concourse.tile as tile
    from concourse import bacc

    return bass, mybir, tile, bacc


def _layout(counts):
    """Padded token layout: each expert's tokens padded to a multiple of P,
    total padded to a multiple of CHUNK. Returns dict with per-expert valid
    counts, padded starts, per-128-tile expert ids."""
    counts = [int(c) for c in counts]
    padded = [(c + P - 1) // P * P for c in counts]
    pstarts = np.concatenate([[0], np.cumsum(padded)]).astype(np.int64)
    tp_valid = int(pstarts[-1])
    Tp = (tp_valid + CHUNK - 1) // CHUNK * CHUNK
    tile_expert = []
    for g, pc in enumerate(padded):
        tile_expert += [g] * (pc // P)
    # tail pad tiles (to reach CHUNK multiple): reuse last expert with tokens
    last_g = max((g for g, c in enumerate(counts) if c > 0), default=0)
    tile_expert += [last_g] * ((Tp - tp_valid) // P)
    return {
        "counts": counts,
        "padded": padded,
        "pstarts": pstarts,
        "Tp": Tp,
        "tile_expert": tile_expert,
    }


def _build_program(tile_expert, KT, dout_shard, mm_dtype_name, n_experts):
    """Build + compile the single-core SPMD Bass/Tile program."""
    bass, mybir, tile, bacc = _concourse()
    hilo = mm_dtype_name == "bf16x3"
    dt_in = mybir.dt.bfloat16 if hilo else getattr(mybir.dt, mm_dtype_name)
    f32 = mybir.dt.float32

    n_tiles = len(tile_expert)
    Tp = n_tiles * P
    assert Tp % CHUNK == 0
    n_chunks = Tp // CHUNK
    TPC = CHUNK // P
    NJ = dout_shard // NSPLIT
    # input streams: (name_suffix,) pairs for hi/lo split or single fp32
    parts = ("h", "l") if hilo else ("",)

    nc = bacc.Bacc("TRN2", target_bir_lowering=False, debug=False)
    x_dram = {p: nc.dram_tensor(f"xt{p}", [n_chunks, KT, P, CHUNK], dt_in,
                                kind="ExternalInput") for p in parts}
    w_dram = {p: nc.dram_tensor(f"wt{p}", [n_experts, KT, P, dout_shard],
                                dt_in, kind="ExternalInput") for p in parts}
    out_dram = nc.dram_tensor("out", [Tp, dout_shard], f32,
                              kind="ExternalOutput")

    with tile.TileContext(nc) as tc:
        with (
            tc.tile_pool(name="wp", bufs=2 * KT * len(parts)) as wp,
            tc.tile_pool(name="xp", bufs=24 * len(parts)) as xp,
            tc.tile_pool(name="op", bufs=3) as op,
            tc.tile_pool(name="pp", bufs=2, space="PSUM") as pp,
        ):
            w_tiles = {}

            def load_w(g):
                tiles = []
                for k in range(KT):
                    tl = {}
                    for p_ in parts:
                        wt = wp.tile([P, dout_shard], dt_in, tag="w",
                                     name=f"w{p_}{g}_{k}")
                        nc.sync.dma_start(out=wt[:], in_=w_dram[p_][g, k])
                        tl[p_] = wt
                    tiles.append(tl)
                w_tiles[g] = tiles

            for c in range(n_chunks):
                xts = []
                for k in range(KT):
                    tl = {}
                    for p_ in parts:
                        xt = xp.tile([P, CHUNK], dt_in, tag="x",
                                     name=f"x{p_}{c}_{k}")
                        nc.sync.dma_start(out=xt[:], in_=x_dram[p_][c, k])
                        tl[p_] = xt
                    xts.append(tl)
                for m in range(TPC):
                    ti = c * TPC + m
                    g = tile_expert[ti]
                    if g not in w_tiles:
                        load_w(g)
                    wts = w_tiles[g]
                    ps = pp.tile([P, dout_shard], f32, tag="ps",
                                 name=f"ps{ti}")
                    for k in range(KT):
                        xk = xts[k]
                        wk = wts[k]
                        ms = m * P
                        if hilo:
                            # stationary-grouped: xh x (wh, wl), then xl x wh
                            ops = [("h", "h"), ("h", "l"), ("l", "h")]
                        else:
                            ops = [("", "")]
                        for oi, (xpart, wpart) in enumerate(ops):
                            for j in range(NJ):
                                nc.tensor.matmul(
                                    ps[:, j * NSPLIT:(j + 1) * NSPLIT],
                                    xk[xpart][:, ms:ms + P],
                                    wk[wpart][:, j * NSPLIT:(j + 1) * NSPLIT],
                                    start=(k == 0 and oi == 0),
                                    stop=(k == KT - 1 and oi == len(ops) - 1),
                                )
                    ot = op.tile([P, dout_shard], f32, tag="o", name=f"o{ti}")
                    nc.vector.tensor_copy(ot[:], ps[:])
                    nc.sync.dma_start(out=out_dram[ti * P:(ti + 1) * P, :],
                                      in_=ot[:])
    nc.compile()
    return nc


def _get_program(lay, KT, dout_shard, n_experts):
    key = (tuple(lay["tile_expert"]), KT, dout_shard, MM_DTYPE, n_experts)
    if key not in _PROG_CACHE:
        _PROG_CACHE[key] = _build_program(lay["tile_expert"], KT, dout_shard,
                                          MM_DTYPE, n_experts)
    return _PROG_CACHE[key]


def _tile_x(xp, KT):
    """padded x [Tp, DIN] -> tiled transposed [n_chunks, KT, P, CHUNK]."""
    Tp, DIN = xp.shape
    n_chunks = Tp // CHUNK
    # [Tp, DIN] -> [c, t, k, p] -> [c, k, p, t]
    xt = xp.reshape(n_chunks, CHUNK, KT, P).transpose(0, 2, 3, 1)
    return np.ascontiguousarray(xt)


def _prepare_x(x, lay, KT):
    """Returns dict of x input arrays keyed by dram tensor name."""
    T, DIN = x.shape
    Tp = lay["Tp"]
    xp = np.zeros((Tp, DIN), dtype=np.float32)
    s = 0
    for g, c in enumerate(lay["counts"]):
        ps = int(lay["pstarts"][g])
        xp[ps:ps + c] = x[s:s + c]
        s += c
    if MM_DTYPE == "bf16x3":
        import ml_dtypes
        bf = ml_dtypes.bfloat16
        xh = xp.astype(bf)
        xl = (xp - xh.astype(np.float32)).astype(bf)
        return {"xth": _tile_x(xh, KT), "xtl": _tile_x(xl, KT)}
    return {"xt": _tile_x(xp, KT)}


def _prepare_w(weight, core, dout_shard, KT):
    """Returns dict of weight shard arrays keyed by dram tensor name."""
    G, DIN, DOUT = weight.shape
    ws = np.ascontiguousarray(
        weight[:, :, core * dout_shard:(core + 1) * dout_shard]
    ).reshape(G, KT, P, dout_shard)
    if MM_DTYPE == "bf16x3":
        import ml_dtypes
        bf = ml_dtypes.bfloat16
        wh = ws.astype(bf)
        wl = (ws - wh.astype(np.float32)).astype(bf)
        return {"wth": wh, "wtl": wl}
    return {"wt": ws}


def _load_ntff_hook():
    """NTFF profiling hook via the axon PJRT plugin's C ABI (the antenv
    axon_hooks module is not shipped in this container)."""
    import importlib.util

    boot_py = "/root/.axon_site/trn_agent_boot/trn_boot.py"
    so_path = "/opt/axon/libaxon_pjrt.so"
    if not (os.path.exists(boot_py) and os.path.exists(so_path)):
        return None
    spec = importlib.util.spec_from_file_location("_trn_boot_mod", boot_py)
    mod = importlib.util.module_from_spec(spec)
    spec.loader.exec_module(mod)
    return mod._ntff_profile_via_ctypes(so_path)


def _run_pjrt(nc, in_maps, n_cores, timing_iters=0):
    """Execute the compiled Bass program on n_cores NeuronCores via PJRT
    (mirrors concourse.bass2jax.run_bass_via_pjrt, but keeps inputs
    device-resident so repeated executions can be timed)."""
    import time

    import jax
    from jax.experimental.shard_map import shard_map
    from jax.sharding import Mesh, NamedSharding, PartitionSpec

    from concourse import bass2jax as b2j
    from concourse import mybir

    b2j.install_neuronx_cc_hook()

    partition_name = (nc.partition_id_tensor.name
                      if nc.partition_id_tensor else None)
    in_names, out_names, out_avals, zero_outs = [], [], [], []
    for alloc in nc.m.functions[0].allocations:
        if not isinstance(alloc, mybir.MemoryLocationSet):
            continue
        name = alloc.memorylocations[0].name
        if alloc.kind == "ExternalInput":
            if name != partition_name:
                in_names.append(name)
        elif alloc.kind == "ExternalOutput":
            out_names.append(name)
            shape = tuple(alloc.tensor_shape)
            dtype = mybir.dt.np(alloc.dtype)
            out_avals.append(jax.core.ShapedArray(shape, dtype))
            zero_outs.append(np.zeros(shape, dtype))
    n_params = len(in_names)
    n_outs = len(out_avals)
    all_in_names = in_names + out_names
    if partition_name is not None:
        all_in_names.append(partition_name)
    donate = tuple(range(n_params, n_params + n_outs))

    def _body(*args):
        operands = list(args)
        if partition_name is not None:
            operands.append(b2j.partition_id_tensor())
        outs = b2j._bass_exec_p.bind(
            *operands,
            out_avals=tuple(out_avals),
            in_names=tuple(all_in_names),
            out_names=tuple(out_names),
            lowering_input_output_aliases=(),
            sim_require_finite=True,
            sim_require_nnan=True,
            nc=nc,
        )
        return tuple(outs)

    devices = jax.devices()[:n_cores]
    assert len(devices) == n_cores
    mesh = Mesh(np.asarray(devices), ("core",))
    spec = NamedSharding(mesh, PartitionSpec("core"))
    sharded = jax.jit(
        shard_map(_body, mesh=mesh,
                  in_specs=(PartitionSpec("core"),) * (n_params + n_outs),
                  out_specs=(PartitionSpec("core"),) * n_outs,
                  check_rep=False),
        donate_argnums=donate,
        keep_unused=True,
    )

    concat_in = [
        jax.device_put(
            np.concatenate([np.asarray(m[name]) for m in in_maps], axis=0),
            spec)
        for name in in_names
    ]
    concat_zeros = [np.zeros((n_cores * z.shape[0], *z.shape[1:]), z.dtype)
                    for z in zero_outs]

    out_arrs = sharded(*concat_in, *[jax.device_put(z, spec)
                                     for z in concat_zeros])
    jax.block_until_ready(out_arrs)
    results = [
        {name: np.asarray(out_arrs[i]).reshape(n_cores, *out_avals[i].shape)[c]
         for i, name in enumerate(out_names)}
        for c in range(n_cores)
    ]

    profile_dir = os.environ.get("KERNEL_PROFILE_DIR")
    if profile_dir:
        hook = _load_ntff_hook()
        if hook is not None:
            with hook(profile_dir, [0]):
                pouts = sharded(*concat_in, *[jax.device_put(z, spec)
                                              for z in concat_zeros])
                jax.block_until_ready(pouts)

    exec_ns = None
    if timing_iters > 0:
        # Donation consumes the zero output buffers, so pre-stage one set
        # per iteration; queue all executions and block once so per-call
        # dispatch latency overlaps device execution.
        zsets = [[jax.device_put(z, spec) for z in concat_zeros]
                 for _ in range(timing_iters)]
        jax.block_until_ready(zsets)
        warm = sharded(*concat_in, *[jax.device_put(z, spec)
                                     for z in concat_zeros])
        jax.block_until_ready(warm)
        t0 = time.perf_counter()
        outs = [sharded(*concat_in, *zs) for zs in zsets]
        jax.block_until_ready(outs)
        t1 = time.perf_counter()
        exec_ns = (t1 - t0) / timing_iters * 1e9
    return results, exec_ns


def _run(x, weight, counts, timing_iters=0):
    if _TRN_REPO not in sys.path:
        sys.path.insert(0, _TRN_REPO)

    x = np.ascontiguousarray(np.asarray(x, dtype=np.float32))
    weight = np.ascontiguousarray(np.asarray(weight, dtype=np.float32))
    counts = np.asarray(counts).astype(np.int64)

    T, DIN = x.shape
    G, DIN2, DOUT = weight.shape
    assert DIN == DIN2 and DIN % P == 0 and int(counts.sum()) == T
    assert DOUT % (N_CORES * NSPLIT) == 0
    KT = DIN // P
    dout_shard = DOUT // N_CORES

    lay = _layout(counts)
    nc = _get_program(lay, KT, dout_shard, G)

    x_arrs = _prepare_x(x, lay, KT)
    in_maps = [{**x_arrs, **_prepare_w(weight, i, dout_shard, KT)}
               for i in range(N_CORES)]

    results, exec_ns = _run_pjrt(nc, in_maps, N_CORES,
                                 timing_iters=timing_iters)
    last_run_info.clear()
    last_run_info["exec_time_ns"] = exec_ns

    out = np.empty((T, DOUT), dtype=np.float32)
    s = 0
    for g, c in enumerate(lay["counts"]):
        ps = int(lay["pstarts"][g])
        for i in range(N_CORES):
            out[s:s + c, i * dout_shard:(i + 1) * dout_shard] = \
                results[i]["out"][ps:ps + c]
        s += c
    return out


def kernel(x, weight, num_inputs_per_group):
    return _run(x, weight, num_inputs_per_group,
                timing_iters=int(os.environ.get("KERNEL_TIMING_ITERS", "0")))
